# revision 1
# baseline (speedup 1.0000x reference)
"""Gaussian-mixture log-likelihood kernel for Trainium2 (8 NeuronCores).

Computes ll[i] = logsumexp_j( wlog[j] - (x_i-mu_j)^T G_j (x_i-mu_j) ),
G_j = A_j A_j^T / 2, wlog = log_softmax(w) + 0.5*log(det(G_j)),
for sample (N,2), mu (M,2), A (M,2,2), w (M,1), N=131072, M=2048.

Strategy: data-parallel over the 8 cores (N/8 = 16384 samples each), the
(M,2,2) parameters replicated.  On each core the pairwise score matrix
v[i,j] = wlog_j - q_ij is a rank-6 matmul:
    phi_i = [x0^2, x0*x1, x1^2, x0, x1, 1]          (6 features / sample)
    theta_j = [-a, -b, -c, 2a*mu0+b*mu1, 2c*mu1+b*mu0, wlog - q(mu)]
with a=G00, b=G01+G10, c=G11.  The PE computes v tile-by-tile
(128 samples x 2048 components) into PSUM, DVE takes the row max,
ACT does exp(v - max) with fused accumulation, and a tiny tail computes
max + log(sum) for all tiles at once.
"""

import os
import sys

import numpy as np

sys.path.insert(0, "/opt/trn_rl_repo")

import concourse.bass as bass
import concourse.bacc as bacc
import concourse.mybir as mybir
from concourse import bass_isa
from concourse.tile import TileContext
from concourse.bass_utils import run_bass_kernel_spmd

N_FULL, M, D = 131072, 2048, 2
NCORES = 8
NSH = N_FULL // NCORES          # samples per core
P = 128                          # partitions
T = NSH // P                     # 128 sample-tiles per core
CPP = M // P                     # 16 components per partition (prep layout)
MM_N = 512                       # moving free dim per matmul
NCHUNK = M // MM_N               # 4 matmuls per tile

f32 = mybir.dt.float32
f32r = mybir.dt.float32r
AF = mybir.ActivationFunctionType
ALU = mybir.AluOpType
AX = mybir.AxisListType
RED = bass_isa.ReduceOp

LOG2 = float(np.log(2.0))


def build_kernel(mm_dtype=f32r, n_tiles=T):
    nsh = n_tiles * P
    nc = bacc.Bacc(
        "TRN2",
        target_bir_lowering=False,
        debug=False,
        num_devices=NCORES,
    )

    sample_e = nc.declare_dram_parameter("sample", [nsh, D], f32, isOutput=False)
    mu_e = nc.declare_dram_parameter("mu", [M, D], f32, isOutput=False)
    A_e = nc.declare_dram_parameter("A", [M, D, D], f32, isOutput=False)
    w_e = nc.declare_dram_parameter("w", [M, 1], f32, isOutput=False)
    out_e = nc.declare_dram_parameter("out", [nsh, 1], f32, isOutput=True)

    with TileContext(nc) as tc:
        with (
            tc.tile_pool(name="singles", bufs=1) as sing,
            tc.tile_pool(name="psum", bufs=2, space="PSUM") as psum,
        ):
            # ---------------- parameter prep: theta (6, M) ----------------
            # component index j = p*CPP + c  (p-major), consistent everywhere
            A_sb = sing.tile([P, CPP * 4], f32, tag="A_sb")
            mu_sb = sing.tile([P, CPP * 2], f32, tag="mu_sb")
            w_sb = sing.tile([P, CPP], f32, tag="w_sb")
            nc.sync.dma_start(
                out=A_sb[:], in_=A_e[:].rearrange("(p c) i j -> p (c i j)", p=P)
            )
            nc.sync.dma_start(
                out=mu_sb[:], in_=mu_e[:].rearrange("(p c) d -> p (c d)", p=P)
            )
            nc.sync.dma_start(
                out=w_sb[:], in_=w_e[:].rearrange("(p c) o -> p (c o)", p=P)
            )

            A4 = A_sb[:].rearrange("p (c e) -> p c e", e=4)
            A00, A01, A10, A11 = (A4[:, :, k] for k in range(4))
            mu2 = mu_sb[:].rearrange("p (c e) -> p c e", e=2)
            mu0, mu1 = mu2[:, :, 0], mu2[:, :, 1]

            def tmp(tag):
                return sing.tile([P, CPP], f32, tag=tag, name=tag)

            th = [sing.tile([P, CPP], f32, tag=f"th{k}", name=f"th{k}") for k in range(6)]
            u = {k: tmp(f"u{k}") for k in range(14)}
            s0, s1, s01 = tmp("s0"), tmp("s1"), tmp("s01")
            det4 = tmp("det4")

            V = nc.vector
            # s0 = A00^2 + A01^2 ; s1 = A10^2 + A11^2 ; s01 = A00*A10 + A01*A11
            V.tensor_tensor(u[0][:], A00, A00, ALU.mult)
            V.tensor_tensor(u[1][:], A01, A01, ALU.mult)
            V.tensor_tensor(s0[:], u[0][:], u[1][:], ALU.add)
            V.tensor_tensor(u[2][:], A10, A10, ALU.mult)
            V.tensor_tensor(u[3][:], A11, A11, ALU.mult)
            V.tensor_tensor(s1[:], u[2][:], u[3][:], ALU.add)
            V.tensor_tensor(u[4][:], A00, A10, ALU.mult)
            V.tensor_tensor(u[5][:], A01, A11, ALU.mult)
            V.tensor_tensor(s01[:], u[4][:], u[5][:], ALU.add)
            # det4 = 4*det(G) = s0*s1 - s01^2
            V.tensor_tensor(u[6][:], s0[:], s1[:], ALU.mult)
            V.tensor_tensor(u[7][:], s01[:], s01[:], ALU.mult)
            V.tensor_tensor(det4[:], u[6][:], u[7][:], ALU.subtract)
            # theta rows 0..2: -a, -b, -c  (a = s0/2, b = s01, c = s1/2)
            V.tensor_scalar(th[0][:], s0[:], -0.5, None, ALU.mult)
            V.tensor_scalar(th[1][:], s01[:], -1.0, None, ALU.mult)
            V.tensor_scalar(th[2][:], s1[:], -0.5, None, ALU.mult)
            # theta row 3 = 2a*mu0 + b*mu1 = s0*mu0 + s01*mu1
            V.tensor_tensor(u[8][:], s0[:], mu0, ALU.mult)
            V.tensor_tensor(u[9][:], s01[:], mu1, ALU.mult)
            V.tensor_tensor(th[3][:], u[8][:], u[9][:], ALU.add)
            # theta row 4 = 2c*mu1 + b*mu0 = s1*mu1 + s01*mu0
            V.tensor_tensor(u[10][:], s1[:], mu1, ALU.mult)
            V.tensor_tensor(u[11][:], s01[:], mu0, ALU.mult)
            V.tensor_tensor(th[4][:], u[10][:], u[11][:], ALU.add)
            # qmu2 = 2*q(mu) = mu0*th3 + mu1*th4
            qmu2 = tmp("qmu2")
            V.tensor_tensor(u[12][:], mu0, th[3][:], ALU.mult)
            V.tensor_tensor(u[13][:], mu1, th[4][:], ALU.mult)
            V.tensor_tensor(qmu2[:], u[12][:], u[13][:], ALU.add)

            # log-softmax denominator of w: lse = log(sum exp(w - max)) + max,
            # computed on a single partition over the (1, M) view of w.
            w_row = sing.tile([1, M], f32, tag="w_row", name="w_row")
            nc.sync.dma_start(
                out=w_row[:], in_=w_e[:].rearrange("(o m) c -> o (m c)", o=1)
            )
            rm = sing.tile([1, 4], f32, tag="rm", name="rm")
            ew_row = sing.tile([1, M], f32, tag="ew_row", name="ew_row")
            V.tensor_reduce(rm[:, 0:1], w_row[:], axis=AX.X, op=ALU.max, negate=True)
            nc.scalar.activation(
                ew_row[:], w_row[:], AF.Exp, bias=rm[:, 0:1], accum_out=rm[:, 1:2]
            )
            nc.scalar.activation(rm[:, 2:3], rm[:, 1:2], AF.Ln)
            # lsew_s = log(sum) - negmax
            V.tensor_tensor(rm[:, 3:4], rm[:, 2:3], rm[:, 0:1], ALU.subtract)
            # broadcast the (1,1) scalar to all partitions via a ones-matmul
            onesrow = sing.tile([1, P], f32, tag="onesrow", name="onesrow")
            V.memset(onesrow[:], 1.0)
            bc_ps = psum.tile([P, M], f32, tag="S", name="bc_ps")
            nc.tensor.matmul(
                bc_ps[:, 0:1], onesrow[:], rm[:, 3:4], start=True, stop=True
            )
            lsew = tmp("lsew")
            V.tensor_copy(lsew[:, 0:1], bc_ps[:, 0:1])

            # theta row 5 = (w - lse) + 0.5*log(det4) - log2 - 0.5*qmu2
            ld = tmp("ld")
            nc.scalar.activation(ld[:], det4[:], AF.Ln)
            w1 = tmp("w1")
            w2 = tmp("w2")
            w3 = tmp("w3")
            V.tensor_scalar(w1[:], w_sb[:], lsew[:, 0:1], None, ALU.subtract)
            V.scalar_tensor_tensor(w2[:], ld[:], 0.5, w1[:], ALU.mult, ALU.add)
            V.scalar_tensor_tensor(w3[:], qmu2[:], -0.5, w2[:], ALU.mult, ALU.add)
            V.tensor_scalar(th[5][:], w3[:], LOG2, None, ALU.subtract)

            # assemble theta (38, M): rows k and 32+k hold theta row k
            # (replicated at base partition 32 for 2-way PE row tiling)
            theta = sing.tile([38, M], f32, tag="theta")
            for k in range(6):
                for off in (0, 32):
                    nc.sync.dma_start(
                        out=theta[off + k : off + k + 1, :].rearrange(
                            "o (p c) -> o p c", p=P
                        ),
                        in_=th[k][:],
                    )

            # ---------------- phi^T (6, nsh) ----------------
            # sample index s = p*n_tiles + j   from the natural row-major load
            x_sb = sing.tile([P, n_tiles * 2], f32, tag="x_sb")
            nc.sync.dma_start(
                out=x_sb[:], in_=sample_e[:].rearrange("(p j) c -> p (j c)", p=P)
            )
            xv = x_sb[:].rearrange("p (j c) -> p j c", c=2)
            x0, x1 = xv[:, :, 0], xv[:, :, 1]

            phisq = sing.tile([P, n_tiles * 3], f32, tag="phisq")
            pv = phisq[:].rearrange("p (j k) -> p j k", k=3)
            V.tensor_tensor(pv[:, :, 0], x0, x0, ALU.mult)
            V.tensor_tensor(pv[:, :, 1], x0, x1, ALU.mult)
            V.tensor_tensor(pv[:, :, 2], x1, x1, ALU.mult)

            ones_sb = sing.tile([P, n_tiles], f32, tag="ones_sb")
            V.memset(ones_sb[:], 1.0)

            phiT = sing.tile([38, nsh], f32, tag="phiT")
            srcs = [pv[:, :, 0], pv[:, :, 1], pv[:, :, 2], x0, x1, ones_sb[:]]
            for k in range(6):
                for off in (0, 32):
                    nc.sync.dma_start(
                        out=phiT[off + k : off + k + 1, :].rearrange(
                            "o (p j) -> o p j", p=P
                        ),
                        in_=srcs[k],
                    )

            # ---------------- main loop ----------------
            s_all = sing.tile([P, n_tiles], f32, tag="s_all")
            nm_all = sing.tile([P, n_tiles], f32, tag="nm_all")

            if mm_dtype == f32:
                theta_mm = theta[:]
                phiT_mm = phiT[:]
            else:
                theta_mm = theta[:].bitcast(mm_dtype)
                phiT_mm = phiT[:].bitcast(mm_dtype)

            assert n_tiles % 2 == 0
            for g in range(n_tiles // 2):
                Ss = []
                for i in (0, 1):
                    Ss.append(psum.tile([P, M], f32, tag="S", name="S"))
                # interleave the two tiles' chunk matmuls so adjacent PE
                # instructions target different 32-row strips (concurrent)
                for cchunk in range(NCHUNK):
                    for i in (0, 1):
                        t = 2 * g + i
                        nc.tensor.matmul(
                            Ss[i][:, cchunk * MM_N : (cchunk + 1) * MM_N],
                            phiT_mm[32 * i : 32 * i + 6, t * P : (t + 1) * P],
                            theta_mm[
                                32 * i : 32 * i + 6,
                                cchunk * MM_N : (cchunk + 1) * MM_N,
                            ],
                            start=True,
                            stop=True,
                            tile_position=(32 * i, 0),
                        )
                for i in (0, 1):
                    t = 2 * g + i
                    # negated row max, then exp(v - max) with fused sum
                    V.tensor_reduce(
                        nm_all[:, t : t + 1],
                        Ss[i][:],
                        axis=AX.X,
                        op=ALU.max,
                        negate=True,
                    )
                    nc.scalar.activation(
                        Ss[i][:],
                        Ss[i][:],
                        AF.Exp,
                        bias=nm_all[:, t : t + 1],
                        accum_out=s_all[:, t : t + 1],
                    )

            # ---------------- tail: ll = log(s) + m ----------------
            ls_all = sing.tile([P, n_tiles], f32, tag="ls_all")
            ll_all = sing.tile([P, n_tiles], f32, tag="ll_all")
            nc.scalar.activation(ls_all[:], s_all[:], AF.Ln)
            V.tensor_tensor(ll_all[:], ls_all[:], nm_all[:], ALU.subtract)
            # sample s = t*P + q lives at ll_all[q, t]
            nc.sync.dma_start(
                out=out_e[:].rearrange("(t p) o -> p (t o)", p=P),
                in_=ll_all[:],
            )

    nc.compile()
    return nc


_NC_CACHE = {}


def _get_nc(mm_dtype_name="float32"):
    if mm_dtype_name not in _NC_CACHE:
        dt = f32r if mm_dtype_name == "float32r" else f32
        _NC_CACHE[mm_dtype_name] = build_kernel(mm_dtype=dt)
    return _NC_CACHE[mm_dtype_name]


def _run(sample, mu, A, w, trace=False, mm_dtype_name="float32"):
    sample = np.ascontiguousarray(np.asarray(sample, dtype=np.float32))
    mu = np.ascontiguousarray(np.asarray(mu, dtype=np.float32))
    A = np.ascontiguousarray(np.asarray(A, dtype=np.float32))
    w = np.ascontiguousarray(np.asarray(w, dtype=np.float32))
    nc = _get_nc(mm_dtype_name)
    shards = np.split(sample, NCORES, axis=0)
    in_maps = [
        {"sample": shards[i], "mu": mu, "A": A, "w": w} for i in range(NCORES)
    ]
    res = run_bass_kernel_spmd(nc, in_maps, list(range(NCORES)), trace=trace)
    out = np.concatenate([res.results[i]["out"] for i in range(NCORES)], axis=0)
    return out.astype(np.float32), res


def kernel(sample, mu, A, w):
    out, _ = _run(sample, mu, A, w, trace=False)
    return out



# revision 6
# speedup vs baseline: 3.7865x; 3.7865x over previous
"""Gaussian-mixture log-likelihood kernel for Trainium2 (8 NeuronCores).

Computes ll[i] = logsumexp_j( wlog[j] - (x_i-mu_j)^T G_j (x_i-mu_j) ),
G_j = A_j A_j^T / 2, wlog = log_softmax(w) + 0.5*log(det(G_j)),
for sample (N,2), mu (M,2), A (M,2,2), w (M,1), N=131072, M=2048.

Data-parallel over 8 cores (16384 samples each), parameters replicated.
Per core the pairwise score v[i,j] = (w_j + 0.5 log det_j - ...) - q_ij is a
rank-6 matmul: phi_i = [x0^2, x0*x1, x1^2, x0, x1, 1] against a (6, M) theta.
The global -logsumexp(w) term is folded into the final output.

Layout: tile j (j=0..127) holds samples {p*128 + j : p}.  phi^T is built
on-chip: features are computed fp32 in natural layout (sample on partition),
split into bf16 hi/lo parts (phi = ph + pl), and packed so tile 4g+l has
rows [ph, ph, pl] (18 rows) at free index g*128 + 32*l + r.  32 bf16 PE
transposes (one per g) move them to partitions 32*l + r, giving 4
concurrent 32-row PE strips.  theta is split the same way into rows
[th_hi, th_lo, th_hi] so one K=18 bf16 matmul computes
ph*th + ph*tl + pl*th = full-fp32-precision scores at bf16 speed.
The main loop double-buffers two (128, 2048) PSUM tiles: PE fills one with
4 N=512 matmuls while DVE takes the row max of the other and ACT computes
exp(v-max) with fused row-sum accumulation.
Tail: ll = log(s) + max - logsumexp(w).
"""

import sys

import numpy as np

sys.path.insert(0, "/opt/trn_rl_repo")

import concourse.bass as bass
import concourse.bacc as bacc
import concourse.mybir as mybir
from concourse import masks
from concourse.tile import TileContext
from concourse.bass_utils import run_bass_kernel_spmd

N_FULL, M, D = 131072, 2048, 2
NCORES = 8
NSH = N_FULL // NCORES          # 16384 samples per core
P = 128
T = NSH // P                     # 128 tiles per core
NGRP = T // 4                    # 32 transpose groups (4 tiles each)
MM_N = 512                       # moving free dim per matmul (1 PSUM bank)
NCHUNK = M // MM_N               # 4 matmuls per tile
PPREP = 32                       # partitions for theta prep math
CPP = M // PPREP                 # 64 components per prep partition

f32 = mybir.dt.float32
f32r = mybir.dt.float32r
bf16 = mybir.dt.bfloat16
AF = mybir.ActivationFunctionType
ALU = mybir.AluOpType
AX = mybir.AxisListType

LOG2 = float(np.log(2.0))


def build_kernel(mm_dtype=f32r):
    nc = bacc.Bacc(
        "TRN2",
        target_bir_lowering=False,
        debug=False,
        num_devices=NCORES,
    )

    sample_e = nc.declare_dram_parameter("sample", [NSH, D], f32, isOutput=False)
    mu_e = nc.declare_dram_parameter("mu", [M, D], f32, isOutput=False)
    A_e = nc.declare_dram_parameter("A", [M, D, D], f32, isOutput=False)
    w_e = nc.declare_dram_parameter("w", [M, 1], f32, isOutput=False)
    out_e = nc.declare_dram_parameter("out", [NSH, 1], f32, isOutput=True)

    with TileContext(nc) as tc:
        with (
            tc.tile_pool(name="sing", bufs=1) as sing,
            tc.tile_pool(name="psum", bufs=2, space="PSUM") as psum,
        ):
            V = nc.vector

            S_a = psum.tile([P, M], f32, tag="S", name="S_a")
            S_b = psum.tile([P, M], f32, tag="S", name="S_b")

            # ---------------- input loads ----------------
            x_sb = sing.tile([P, T * 2], f32, tag="x_sb")
            x_hbm = sample_e[:].rearrange("(p j) c -> p (j c)", p=P)
            for c in range(4):
                nc.sync.dma_start(
                    out=x_sb[32 * c : 32 * (c + 1), :],
                    in_=x_hbm[32 * c : 32 * (c + 1), :],
                )

            A_sb = sing.tile([PPREP, CPP * 4], f32, tag="A_sb")
            mu_sb = sing.tile([PPREP, CPP * 2], f32, tag="mu_sb")
            w_sb = sing.tile([PPREP, CPP], f32, tag="w_sb")
            nc.sync.dma_start(
                out=A_sb[:], in_=A_e[:].rearrange("(p c) i j -> p (c i j)", p=PPREP)
            )
            nc.sync.dma_start(
                out=mu_sb[:], in_=mu_e[:].rearrange("(p c) d -> p (c d)", p=PPREP)
            )
            nc.sync.dma_start(
                out=w_sb[:], in_=w_e[:].rearrange("(p c) o -> p (c o)", p=PPREP)
            )
            w_row = sing.tile([1, M], f32, tag="w_row")
            nc.sync.dma_start(
                out=w_row[:], in_=w_e[:].rearrange("(o m) c -> o (m c)", o=1)
            )

            # ---------------- w log-softmax denominator ----------------
            # lsew = logsumexp(w); broadcast to all partitions via ones-matmul
            rm = sing.tile([1, 4], f32, tag="rm")
            ew_row = sing.tile([1, M], f32, tag="ew_row")
            V.tensor_reduce(rm[:, 0:1], w_row[:], axis=AX.X, op=ALU.max, negate=True)
            nc.scalar.activation(
                ew_row[:], w_row[:], AF.Exp, bias=rm[:, 0:1], accum_out=rm[:, 1:2]
            )
            nc.scalar.activation(rm[:, 2:3], rm[:, 1:2], AF.Ln)
            V.tensor_tensor(rm[:, 3:4], rm[:, 2:3], rm[:, 0:1], ALU.subtract)
            onesrow = sing.tile([1, P], f32, tag="onesrow")
            V.memset(onesrow[:], 1.0)
            nc.tensor.matmul(
                S_a[:, 0:1], onesrow[:], rm[:, 3:4], start=True, stop=True
            )
            lsew = sing.tile([P, 1], f32, tag="lsew")
            V.tensor_copy(lsew[:], S_a[:, 0:1])

            # ---------------- theta prep on (32, 64) ----------------
            A4 = A_sb[:].rearrange("p (c e) -> p c e", e=4)
            A00, A01, A10, A11 = (A4[:, :, k] for k in range(4))
            mu2 = mu_sb[:].rearrange("p (c e) -> p c e", e=2)
            mu0, mu1 = mu2[:, :, 0], mu2[:, :, 1]

            def tmp(tag):
                return sing.tile([PPREP, CPP], f32, tag=tag, name=tag)

            th = [tmp(f"th{k}") for k in range(6)]
            u = [tmp(f"u{k}") for k in range(4)]
            s0, s1, s01, det4, qmu2 = (
                tmp("s0"), tmp("s1"), tmp("s01"), tmp("det4"), tmp("qmu2")
            )

            V.tensor_tensor(u[0][:], A00, A00, ALU.mult)
            V.tensor_tensor(u[1][:], A01, A01, ALU.mult)
            V.tensor_tensor(s0[:], u[0][:], u[1][:], ALU.add)
            V.tensor_tensor(u[2][:], A10, A10, ALU.mult)
            V.tensor_tensor(u[3][:], A11, A11, ALU.mult)
            V.tensor_tensor(s1[:], u[2][:], u[3][:], ALU.add)
            V.tensor_tensor(u[0][:], A00, A10, ALU.mult)
            V.tensor_tensor(u[1][:], A01, A11, ALU.mult)
            V.tensor_tensor(s01[:], u[0][:], u[1][:], ALU.add)
            V.tensor_tensor(u[2][:], s0[:], s1[:], ALU.mult)
            V.tensor_tensor(u[3][:], s01[:], s01[:], ALU.mult)
            V.tensor_tensor(det4[:], u[2][:], u[3][:], ALU.subtract)
            V.tensor_scalar(th[0][:], s0[:], -0.5, None, ALU.mult)
            V.tensor_scalar(th[1][:], s01[:], -1.0, None, ALU.mult)
            V.tensor_scalar(th[2][:], s1[:], -0.5, None, ALU.mult)
            V.tensor_tensor(u[0][:], s0[:], mu0, ALU.mult)
            V.tensor_tensor(u[1][:], s01[:], mu1, ALU.mult)
            V.tensor_tensor(th[3][:], u[0][:], u[1][:], ALU.add)
            V.tensor_tensor(u[2][:], s1[:], mu1, ALU.mult)
            V.tensor_tensor(u[3][:], s01[:], mu0, ALU.mult)
            V.tensor_tensor(th[4][:], u[2][:], u[3][:], ALU.add)
            V.tensor_tensor(u[0][:], mu0, th[3][:], ALU.mult)
            V.tensor_tensor(u[1][:], mu1, th[4][:], ALU.mult)
            V.tensor_tensor(qmu2[:], u[0][:], u[1][:], ALU.add)
            ld = tmp("ld")
            nc.scalar.activation(ld[:], det4[:], AF.Ln)
            w2 = tmp("w2")
            w3 = tmp("w3")
            V.scalar_tensor_tensor(w2[:], ld[:], 0.5, w_sb[:], ALU.mult, ALU.add)
            V.scalar_tensor_tensor(w3[:], qmu2[:], -0.5, w2[:], ALU.mult, ALU.add)
            V.tensor_scalar(th[5][:], w3[:], LOG2, None, ALU.subtract)

            # bf16 hi/lo split of theta rows
            th_h = [
                sing.tile([PPREP, CPP], bf16, tag=f"thh{k}", name=f"thh{k}")
                for k in range(6)
            ]
            th_l = [
                sing.tile([PPREP, CPP], bf16, tag=f"thl{k}", name=f"thl{k}")
                for k in range(6)
            ]
            for k in range(6):
                V.tensor_copy(th_h[k][:], th[k][:])
                V.tensor_tensor(th_l[k][:], th[k][:], th_h[k][:], ALU.subtract)

            # theta (128, M) bf16: per strip q, rows 32q+[0:6)=hi,
            # [6:12)=lo, [12:18)=hi  (pairs with phi rows [ph, ph, pl])
            theta = sing.tile([P, M], bf16, tag="theta")
            for k in range(6):
                for r, t in ((0, th_h[k]), (6, th_l[k]), (12, th_h[k])):
                    nc.sync.dma_start(
                        out=theta[r + k : r + k + 1, :].rearrange(
                            "o (p c) -> o p c", p=PPREP
                        ),
                        in_=t[:],
                    )
            for q in range(1, 4):
                nc.sync.dma_start(
                    out=theta[32 * q : 32 * q + 18, :], in_=theta[0:18, :]
                )

            # ---------------- phi features, natural layout ----------------
            # phinat[p, g*128 + 32*l + (k 0..5)] = feature k of sample
            # p*128 + 4g + l, computed in fp32
            phinat = sing.tile([P, NGRP * P], f32, tag="phinat")
            pv4 = phinat[:].rearrange("p (g l k) -> p g l k", g=NGRP, l=4)
            pv = pv4[:, :, :, 0:6]
            xv = x_sb[:].rearrange("p (g l c) -> p g l c", g=NGRP, l=4)
            x0, x1 = xv[:, :, :, 0], xv[:, :, :, 1]
            V.tensor_tensor(pv[:, :, :, 0], x0, x0, ALU.mult)
            V.tensor_tensor(pv[:, :, :, 1], x0, x1, ALU.mult)
            V.tensor_tensor(pv[:, :, :, 2], x1, x1, ALU.mult)
            V.tensor_copy(pv[:, :, :, 3], x0)
            V.tensor_copy(pv[:, :, :, 4], x1)
            V.memset(pv[:, :, :, 5], 1.0)

            # bf16 split, packed: rows [0:6)=hi, [6:12)=hi, [12:18)=lo
            phb = sing.tile([P, NGRP * P], bf16, tag="phb")
            pb4 = phb[:].rearrange("p (g l k) -> p g l k", g=NGRP, l=4)
            V.tensor_copy(pb4[:, :, :, 0:6], pv)
            V.tensor_copy(pb4[:, :, :, 6:12], pv)
            V.tensor_tensor(pb4[:, :, :, 12:18], pv, pb4[:, :, :, 0:6], ALU.subtract)

            # ---------------- PE transposes -> phiT ----------------
            ident = sing.tile([P, P], bf16, tag="ident")
            masks.make_identity(nc, ident[:])

            phiT = sing.tile([P, NGRP * P], bf16, tag="phiT")
            S_a_bf = S_a[:].bitcast(bf16)
            for g in range(NGRP):
                nc.tensor.transpose(
                    S_a_bf[:, g * P : (g + 1) * P],
                    phb[:, g * P : (g + 1) * P],
                    ident[:],
                )
            V.tensor_copy(phiT[:], S_a_bf[:])

            # ---------------- main loop ----------------
            s_all = sing.tile([P, T], f32, tag="s_all")
            nm_all = sing.tile([P, T], f32, tag="nm_all")
            dum = sing.tile([P, M], f32, tag="dum")

            theta_mm = theta[:]
            phiT_mm = phiT[:]
            del mm_dtype

            for j in range(T):
                g, l = j // 4, j % 4
                S = S_a if j % 2 == 0 else S_b
                for c in range(NCHUNK):
                    nc.tensor.matmul(
                        S[:, c * MM_N : (c + 1) * MM_N],
                        phiT_mm[32 * l : 32 * l + 18, g * P : (g + 1) * P],
                        theta_mm[32 * l : 32 * l + 18, c * MM_N : (c + 1) * MM_N],
                        start=True,
                        stop=True,
                        tile_position=(32 * l, 0),
                    )
                V.tensor_reduce(
                    nm_all[:, j : j + 1], S[:], axis=AX.X, op=ALU.max, negate=True
                )
                nc.scalar.activation(
                    dum[:],
                    S[:],
                    AF.Exp,
                    bias=nm_all[:, j : j + 1],
                    accum_out=s_all[:, j : j + 1],
                )

            # ---------------- tail: ll = log(s) + m - lsew ----------------
            ls_all = sing.tile([P, T], f32, tag="ls_all")
            ll_all = sing.tile([P, T], f32, tag="ll_all")
            ll2 = sing.tile([P, T], f32, tag="ll2")
            nc.scalar.activation(ls_all[:], s_all[:], AF.Ln)
            V.tensor_tensor(ll_all[:], ls_all[:], nm_all[:], ALU.subtract)
            V.tensor_scalar(ll2[:], ll_all[:], lsew[:, 0:1], None, ALU.subtract)
            out_hbm = out_e[:].rearrange("(p j) o -> p (j o)", p=P)
            for c in range(4):
                nc.sync.dma_start(
                    out=out_hbm[32 * c : 32 * (c + 1), :],
                    in_=ll2[32 * c : 32 * (c + 1), :],
                )

    nc.compile()
    return nc


_NC_CACHE = {}


def _get_nc(mm_dtype_name="float32r"):
    if mm_dtype_name not in _NC_CACHE:
        dt = f32r if mm_dtype_name == "float32r" else f32
        _NC_CACHE[mm_dtype_name] = build_kernel(mm_dtype=dt)
    return _NC_CACHE[mm_dtype_name]


def _run(sample, mu, A, w, trace=False, mm_dtype_name="float32r"):
    sample = np.ascontiguousarray(np.asarray(sample, dtype=np.float32))
    mu = np.ascontiguousarray(np.asarray(mu, dtype=np.float32))
    A = np.ascontiguousarray(np.asarray(A, dtype=np.float32))
    w = np.ascontiguousarray(np.asarray(w, dtype=np.float32))
    nc = _get_nc(mm_dtype_name)
    shards = np.split(sample, NCORES, axis=0)
    in_maps = [
        {"sample": shards[i], "mu": mu, "A": A, "w": w} for i in range(NCORES)
    ]
    res = run_bass_kernel_spmd(nc, in_maps, list(range(NCORES)), trace=trace)
    out = np.concatenate([res.results[i]["out"] for i in range(NCORES)], axis=0)
    return out.astype(np.float32), res


def kernel(sample, mu, A, w):
    out, _ = _run(sample, mu, A, w, trace=False)
    return out
